# revision 18
# baseline (speedup 1.0000x reference)
"""Trainium2 Bass kernel for nn_ConcatHeadModule (pairwise concat-head scorer).

Math (reference):
    xc   = x.reshape(T, 2L)
    actH = tanh(xc @ W1H + cbH);  actM = tanh(xc @ W1M + cbM)
    AH   = actH @ L2H;            AM   = actM @ L2M
    scores[i,j] = sum_h wOut[h]*tanh(AH[i,h] + AM[j,h] + h2b[h]) + outBias

Instead of evaluating the T*T*H2 pairwise tanh on ScalarE (the baseline's
bottleneck: ~250us of ACTIVATE per core), tanh is expanded in an odd
harmonic sine series fitted offline:

    tanh(s) ~= sum_k g_k * sin((2k-1)*w0*s),   k = 1..K

Each term is separable across the pairwise sum s = a_i + B_j:

    sin(w(a+B)) = sin(wa)cos(wB) + cos(wa)sin(wB)

so the score matrix becomes a single TensorE contraction over (k, h):

    scores[i,j] = sum_k sum_h [g_k*w_h*sin(w_k a)]*cos(w_k B)
                            + [g_k*w_h*cos(w_k a)]*sin(w_k B)

Only the base streams sin(w0 v), cos(w0 v), cos(2 w0 v) are evaluated with
ScalarE's Sin table (all arguments stay inside its valid [-pi, pi] domain,
read straight out of the AM/AH PSUM tiles); higher harmonics come from the
3-term recurrence  s_{k+1} = 2*cos(2 w0 v)*s_k - s_{k-1}  on the vector
engine, in bf16 tensor_tensor ops (2x perf mode), with the sin and cos
chains sharing tiles. Sharding: rows of the score grid, 96 per core.

All shapes are hardcoded (T=768, 2L=512, HID=512, HID2=512, 8 cores).
"""

import os
import sys

for _p in ("/root/.axon_site", "/root/.axon_site/_ro/trn_rl_repo", "/opt/trn_rl_repo"):
    if os.path.isdir(_p) and _p not in sys.path:
        sys.path.append(_p)

import math

import ml_dtypes
import numpy as np

import concourse.bass as bass
import concourse.mybir as mybir
import concourse.tile as tile
from concourse import bacc
from concourse.bass_utils import run_bass_kernel_spmd

F32 = mybir.dt.float32
BF16 = mybir.dt.bfloat16
TANH = mybir.ActivationFunctionType.Tanh
SIN = mybir.ActivationFunctionType.Sin
IDENT = mybir.ActivationFunctionType.Identity
MULT = mybir.AluOpType.mult
ADD = mybir.AluOpType.add
SUB = mybir.AluOpType.subtract

T = 768          # tokens
C = 512          # 2 * LDIMS (concat lstm state)
H = 512          # hidden1
H2 = 512         # hidden2
NCORES = 8
R = T // NCORES  # score rows per core
P = 128          # partitions
NK = 4           # 128-chunks in C/H/H2
TH = T // 2      # 384, psum half-width for j

# Odd-harmonic sine expansion of tanh: tanh(s) ~= sum g[k] sin((2k+1) W0 s)
W0 = 0.3560
GAMMA = [1.209316, 0.275573, 0.087332, 0.031753]
K = len(GAMMA)


def build_nc(rows: int = R) -> bass.Bass:
    nc = bacc.Bacc("TRN2", target_bir_lowering=False, num_devices=NCORES)

    xT = nc.dram_tensor("xT", [C, T], BF16, kind="ExternalInput")
    xTi = nc.dram_tensor("xTi", [C, rows], BF16, kind="ExternalInput")
    w1m = nc.dram_tensor("w1m", [C, H], BF16, kind="ExternalInput")
    w1h = nc.dram_tensor("w1h", [C, H], BF16, kind="ExternalInput")
    l2m = nc.dram_tensor("l2m", [H, H2], BF16, kind="ExternalInput")
    l2h = nc.dram_tensor("l2h", [H, H2], BF16, kind="ExternalInput")
    # cb columns: cbm[4], cbh[4], ub1[4]=W0*h2b, ub2[4]=W0*h2b+pi/2,
    #             ub3[4]=2*W0*h2b+pi/2, pihalf[1], ob[1]
    cb = nc.dram_tensor("cb", [P, 5 * NK + 2], F32, kind="ExternalInput")
    wexp = nc.dram_tensor("wexp", [P, 2 * NK * R], BF16, kind="ExternalInput")
    out_rows = nc.dram_tensor("out_rows", [rows, T], F32, kind="ExternalOutput")

    with tile.TileContext(nc) as tc:
        _emit(tc, locals(), rows)
    nc.compile()
    return nc


def _emit(tc: tile.TileContext, io, rows: int):
    nc = tc.nc
    xT, xTi, w1m, w1h = io["xT"], io["xTi"], io["w1m"], io["w1h"]
    l2m, l2h, cb, wexp = io["l2m"], io["l2h"], io["cb"], io["wexp"]
    out_rows = io["out_rows"]
    UW = 2 * NK * rows   # 768: [sin 4hc | cos 4hc] U tile width
    HUW = UW // 2        # 384
    VB = 2 * T           # 1536: per-hc V block [sin 768 | cos 768]
    VW = NK * VB         # 6144
    HVW = VW // 2        # 3072

    with tc.tile_pool(name="const", bufs=1) as const:
        setup_cm = tc.tile_pool(name="setup_sb", bufs=1)
        setup = setup_cm.__enter__()

        # ---- input DMAs (few, coalesced; alternate the two HWDGE queues) ----
        cb_all = const.tile([P, 5 * NK + 2], F32, name="cb_all")
        nc.sync.dma_start(cb_all[:], cb[:, :])
        cbm_sb = [cb_all[:, k:k + 1] for k in range(NK)]
        cbh_sb = [cb_all[:, NK + k:NK + k + 1] for k in range(NK)]
        ub_sb = [[cb_all[:, (1 + v) * NK + k:(1 + v) * NK + k + 1]
                  for k in range(NK)] for v in (1, 2, 3)]
        pihalf = cb_all[:, 5 * NK:5 * NK + 1]
        ob_sb = cb_all[:, 5 * NK + 1:5 * NK + 2]
        wexp_sb = const.tile([P, UW], BF16, name="wexp_sb")
        nc.sync.dma_start(wexp_sb[:], wexp[:, :])

        def load_coalesced(name, dram, k, inner, eng):
            t = setup.tile([P, k * inner], BF16, name=name)
            eng.dma_start(
                t[:].rearrange("p (k t) -> p k t", k=k),
                dram[:].rearrange("(k p) t -> p k t", p=P),
            )
            return t

        # one coalesced DMA per tensor: SP-queue issue is ~0.7us per DMA,
        # so 6 big DMAs beat 19 small ones; l2m ahead of w1h on the ACT
        # queue because the AM matmuls need it first
        w1m_sb = load_coalesced("w1m_sb", w1m, NK, H, nc.scalar)
        xT_sb = load_coalesced("xT_sb", xT, NK, T, nc.sync)
        l2m_sb = load_coalesced("l2m_sb", l2m, NK, H2, nc.scalar)
        xTi_sb = load_coalesced("xTi_sb", xTi, NK, rows, nc.sync)
        w1h_sb = load_coalesced("w1h_sb", w1h, NK, H, nc.scalar)
        l2h_sb = load_coalesced("l2h_sb", l2h, NK, H2, nc.sync)

        setup_ps_cm = tc.tile_pool(name="setup_ps", bufs=6, space="PSUM")
        setup_ps = setup_ps_cm.__enter__()

        # ---- actM^T = tanh(W1M^T @ xc^T + cbm), bf16 [h1-chunk | j] ----
        # cc is the OUTER loop over up to 6 psum banks (waves of 3 hc):
        # each xc chunk is consumed by every group in the wave the moment
        # its DMA lands, so the groups finish (and tanh streams) right
        # after the last chunk arrives
        actMT = setup.tile([P, NK * T], BF16, name="actMT")
        for hc in range(NK):
            for b, n0 in enumerate((0, TH)):
                ps = setup_ps.tile([P, TH], F32, tag="setup",
                                   name=f"mf{hc}_{b}")
                for cc in range(NK):
                    nc.tensor.matmul(
                        ps[:],
                        lhsT=w1m_sb[:, cc * H + hc * P:cc * H + (hc + 1) * P],
                        rhs=xT_sb[:, cc * T + n0:cc * T + n0 + TH],
                        start=(cc == 0),
                        stop=(cc == NK - 1),
                    )
                nc.scalar.activation(
                    actMT[:, hc * T + n0:hc * T + n0 + TH], ps[:], TANH,
                    bias=cbm_sb[hc][:],
                )

        # ---- H side: actH^T, then a = AH^T + h2b folded into ACT biases ----
        actHT = setup.tile([P, NK * rows], BF16, name="actHT")
        for hc in range(NK):
            ps = setup_ps.tile([P, rows], F32, tag="setup")
            for cc in range(NK):
                nc.tensor.matmul(
                    ps[:],
                    lhsT=w1h_sb[:, cc * H + hc * P:cc * H + (hc + 1) * P],
                    rhs=xTi_sb[:, cc * rows:(cc + 1) * rows],
                    start=(cc == 0),
                    stop=(cc == NK - 1),
                )
            nc.scalar.activation(
                actHT[:, hc * rows:(hc + 1) * rows], ps[:], TANH,
                bias=cbh_sb[hc][:],
            )

        # U base streams straight from the AH psum:
        #   sin(W0 a) = Sin(W0*AH + W0*h2b), cos via +pi/2, q = cos(2 W0 a)
        u1raw = setup.tile([P, UW], BF16, name="u1raw")
        qu = setup.tile([P, HUW], BF16, name="qu")
        for hc in range(NK):
            ps = setup_ps.tile([P, rows], F32, tag="setup")
            for kc in range(NK):
                nc.tensor.matmul(
                    ps[:],
                    lhsT=l2h_sb[:, kc * H2 + hc * P:kc * H2 + (hc + 1) * P],
                    rhs=actHT[:, kc * rows:(kc + 1) * rows],
                    start=(kc == 0),
                    stop=(kc == NK - 1),
                )
            sl = slice(hc * rows, (hc + 1) * rows)
            nc.scalar.activation(u1raw[:, sl], ps[:], SIN,
                                 scale=W0, bias=ub_sb[0][hc][:])
            nc.scalar.activation(u1raw[:, HUW + hc * rows:HUW + (hc + 1) * rows],
                                 ps[:], SIN, scale=W0, bias=ub_sb[1][hc][:])
            nc.scalar.activation(qu[:, sl], ps[:], SIN,
                                 scale=2 * W0, bias=ub_sb[2][hc][:])

        # U chains (bf16, w-scaled seeds so wOut propagates for free)
        SCu = [setup.tile([P, UW], BF16, name=f"SCu{k}") for k in range(K)]
        USC = [const.tile([P, UW], BF16, name=f"USC{k}") for k in range(K)]
        nc.vector.tensor_tensor(SCu[0][:], u1raw[:], wexp_sb[:], MULT)
        nc.vector.tensor_scalar(USC[0][:], SCu[0][:], float(GAMMA[0]),
                                None, MULT)
        mu = setup.tile([P, UW], BF16, name="mu")
        mpmu = setup.tile([P, UW], BF16, name="mpmu")
        nc.vector.tensor_scalar(mpmu[:, :HUW], qu[:], 2.0, 1.0, MULT, ADD)
        nc.vector.tensor_scalar(mpmu[:, HUW:], qu[:], 2.0, -1.0, MULT, ADD)
        nc.vector.tensor_tensor(SCu[1][:], mpmu[:], SCu[0][:], MULT)
        nc.vector.tensor_scalar(USC[1][:], SCu[1][:], float(GAMMA[1]),
                                None, MULT)
        nc.vector.tensor_scalar(mu[:, :HUW], qu[:], 2.0, None, MULT)
        nc.vector.tensor_scalar(mu[:, HUW:], qu[:], 2.0, None, MULT)
        tu = setup.tile([P, UW], BF16, name="tu")
        for k in range(2, K):
            nc.vector.tensor_tensor(tu[:], mu[:], SCu[k - 1][:], MULT)
            nc.vector.tensor_tensor(SCu[k][:], tu[:], SCu[k - 2][:], SUB)
            nc.vector.tensor_scalar(USC[k][:], SCu[k][:], float(GAMMA[k]),
                                    None, MULT)

        # ---- V side: AM^T psum -> base trig directly, hc-major layout ----
        # SCv[k] blocks: [hc0: sin(768)|cos(768)] [hc1: ...] ...
        qv = const.tile([P, HVW], BF16, name="qv")  # hc-major cos(2 W0 B)
        SCv = [const.tile([P, VW], BF16, name=f"SCv{k}") for k in range(K)]
        mv = const.tile([P, VW], BF16, name="mv")
        mpmv = const.tile([P, VW], BF16, name="mpmv")
        # kc OUTER (waves of 3 hc): AM matmuls for chunk kc begin as soon
        # as tanh(chunk kc) lands, overlapping the other M-side activations
        for wave in ((0, 1), (2, 3)):
            aps = {(hc, b): setup_ps.tile([P, TH], F32, tag="setup",
                                          name=f"am{hc}_{b}")
                   for hc in wave for b in range(2)}
            for kc in range(NK):
                for hc in wave:
                    for b, n0 in enumerate((0, TH)):
                        nc.tensor.matmul(
                            aps[hc, b][:],
                            lhsT=l2m_sb[:, kc * H2 + hc * P:
                                        kc * H2 + (hc + 1) * P],
                            rhs=actMT[:, kc * T + n0:kc * T + n0 + TH],
                            start=(kc == 0),
                            stop=(kc == NK - 1),
                        )
            for hc in wave:
                for b, n0 in enumerate((0, TH)):
                    ps = aps[hc, b]
                    nc.scalar.activation(
                        SCv[0][:, hc * VB + n0:hc * VB + n0 + TH], ps[:], SIN,
                        scale=W0)
                    nc.scalar.activation(
                        SCv[0][:, hc * VB + T + n0:hc * VB + T + n0 + TH],
                        ps[:], SIN, scale=W0, bias=pihalf[:])
                    nc.scalar.activation(
                        qv[:, hc * T + n0:hc * T + n0 + TH], ps[:], SIN,
                        scale=2 * W0, bias=pihalf[:])

        # ---- main loop: recurrence halves interleaved with the matmuls ----
        # (recurrence in 2-hc halves; TensorE consumes each half of stream k
        # while the DVE produces the next, keeping PE-idle gaps below the
        # ~3.4us HAM re-throttle window)
        tv = setup.tile([P, VW], BF16, name="tv")
        with tc.tile_pool(name="row_ps", bufs=2, space="PSUM") as row_ps:
            psr = [row_ps.tile([rows, TH], F32, tag="row", name=f"psr{b}")
                   for b in range(2)]
            nmm = K * NK * 2
            idx = 0
            def build_mult(dst, half, s2):
                for hc in (2 * half, 2 * half + 1):
                    qb = qv[:, hc * T:(hc + 1) * T]
                    nc.vector.tensor_scalar(dst[:, hc * VB:hc * VB + T], qb,
                                            2.0, s2, MULT, ADD)
                    nc.vector.tensor_scalar(dst[:, hc * VB + T:(hc + 1) * VB],
                                            qb, 2.0, -s2 if s2 else s2,
                                            MULT, ADD)

            for k in range(K):
                for half in (0, 1):
                    hs = slice(half * HVW, (half + 1) * HVW)
                    if k == 1:
                        build_mult(mpmv, half, 1.0)
                        nc.vector.tensor_tensor(SCv[1][:, hs], mpmv[:, hs],
                                                SCv[0][:, hs], MULT)
                    elif k == 2:
                        build_mult(mv, half, 0.0)
                        nc.vector.tensor_tensor(tv[:, hs], mv[:, hs],
                                                SCv[k - 1][:, hs], MULT)
                        nc.vector.tensor_tensor(SCv[k][:, hs], tv[:, hs],
                                                SCv[k - 2][:, hs], SUB)
                    elif k >= 3:
                        nc.vector.tensor_tensor(tv[:, hs], mv[:, hs],
                                                SCv[k - 1][:, hs], MULT)
                        nc.vector.tensor_tensor(SCv[k][:, hs], tv[:, hs],
                                                SCv[k - 2][:, hs], SUB)
                    for hc in (2 * half, 2 * half + 1):
                        # (U sin, V cos), (U cos, V sin)
                        for (u0, v0) in ((0, hc * VB + T), (HUW, hc * VB)):
                            idx += 1
                            for b, n0 in enumerate((0, TH)):
                                nc.tensor.matmul(
                                    psr[b][:],
                                    lhsT=USC[k][:, u0 + hc * rows:
                                                u0 + (hc + 1) * rows],
                                    rhs=SCv[k][:, v0 + n0:v0 + n0 + TH],
                                    start=(idx == 1),
                                    stop=(idx == nmm),
                                )
            ev = const.tile([P, T], F32, name="ev")
            for b, n0 in enumerate((0, TH)):
                nc.scalar.activation(ev[0:rows, n0:n0 + TH], psr[b][:], IDENT,
                                     bias=ob_sb[0:rows, :])
            nc.sync.dma_start(out_rows[:, :], ev[0:rows, :])

        setup_ps_cm.__exit__(None, None, None)
        setup_cm.__exit__(None, None, None)


def _prep_inputs(x, hidLayerFOH, hidLayerFOM, catBias, hid2Layer, hid2Bias,
                 outLayer, outBias, rows=R, ncores=NCORES):
    """Host-side layout prep (reshape/transpose/slice/cast only)."""
    bf = ml_dtypes.bfloat16
    x = np.asarray(x, np.float32)
    xc = x.reshape(T, C)
    wout = np.asarray(outLayer, np.float32).reshape(NK, P).T  # [128, 4]
    wexp = np.tile(np.repeat(wout, rows, axis=1), (1, 2))     # [128, 768]
    h2b = np.asarray(hid2Bias, np.float32).reshape(NK, P).T
    cb_all = np.concatenate([
        np.asarray(catBias[H:], np.float32).reshape(NK, P).T,
        np.asarray(catBias[:H], np.float32).reshape(NK, P).T,
        W0 * h2b,
        W0 * h2b + math.pi / 2,
        2 * W0 * h2b + math.pi / 2,
        np.full((P, 1), math.pi / 2, np.float32),
        np.full((P, 1), np.asarray(outBias, np.float32).reshape(()), np.float32),
    ], axis=1).astype(np.float32)
    common = {
        "xT": np.ascontiguousarray(xc.T).astype(bf),
        "w1m": np.asarray(hidLayerFOM, np.float32).astype(bf),
        "w1h": np.asarray(hidLayerFOH, np.float32).astype(bf),
        "l2m": np.asarray(hid2Layer, np.float32)[H:].astype(bf),
        "l2h": np.asarray(hid2Layer, np.float32)[:H].astype(bf),
        "cb": np.ascontiguousarray(cb_all),
        "wexp": np.ascontiguousarray(wexp).astype(bf),
    }
    in_maps = []
    for c in range(ncores):
        m = dict(common)
        m["xTi"] = np.ascontiguousarray(
            xc[c * rows:(c + 1) * rows].T).astype(bf)
        in_maps.append(m)
    return in_maps


def kernel(x, hidLayerFOH, hidLayerFOM, catBias, hid2Layer, hid2Bias,
           outLayer, outBias, _trace=False):
    in_maps = _prep_inputs(x, hidLayerFOH, hidLayerFOM, catBias,
                           hid2Layer, hid2Bias, outLayer, outBias)
    nc = build_nc(R)
    res = run_bass_kernel_spmd(nc, in_maps, core_ids=list(range(NCORES)),
                               trace=_trace)
    out = np.concatenate([res.results[c]["out_rows"] for c in range(NCORES)], 0)
    if _trace:
        kernel.last_results = res
    return out.astype(np.float32)
